# revision 1
# baseline (speedup 1.0000x reference)
"""Single-head attention kernel for TRN2, 8 NeuronCores.

Problem: hidden [4,4096,1024] fp32; Wq/Wk/Wv [1024,64]; out [4,4096,64]
  q,k,v = hidden @ W + b ; out = softmax(q k^T / 8) @ v

Sharding: 2 cores per batch; each core handles 2048 query rows but computes
K/V for the full 4096-row sequence of its batch (sequence parallelism over
the Q rows, K/V recomputed per core — no collectives). Host-side prep per
core: hidden[b] is rotated so this core's query rows are rows 0:2048, then
transposed to hidT [1024, 4096] (E on partitions — required because the PE
contracts along the partition dim). Softmax over keys is permutation-
invariant, so the rotation does not change results.

On-chip layout (all matmul operands float32r = TF32-like, 1 cyc/row):
  kT   [128, 4096]  rows 0:64 = k^T, rows 64:128 = copy (scores row-packing)
  qT   [128, 2, 512] pair P: rows 0:64 = q^T cols 1024P+0:512,
                              rows 64:128 = q^T cols 1024P+512:1024
  vones[128, 32, 65] per s_k tile: cols 0:64 = v natural, col 64 = 1.0
                     (the ones column makes AV also produce softmax sums)
  scores^T tile: psum [128, 1024] = row-packed pair (A, B) of [128, 512]
  exp: ACT, scale=1/8 folded, no max-subtraction (scores ~N(0,1), safe)
  AV:  out^T accum psum [65, 512] per s_q block; row 64 = denominators
  epilogue: transpose out^T -> [128, 65], multiply by reciprocal(row sums)
"""

import numpy as np

E, S, H = 1024, 4096, 64
NT = E // 128  # 8 e-tiles
SQ = S // 2  # 2048 query rows per core
NK = S // 128  # 32 s_k tiles
N_CORES = 8

_NC = None
LAST_RESULT = None  # BassKernelResults of the most recent run (for test.py)


def _build():
    from contextlib import ExitStack
    import concourse.tile as tile
    from concourse import bacc, mybir
    from concourse.masks import make_identity

    F32 = mybir.dt.float32
    F32R = mybir.dt.float32r
    Exp = mybir.ActivationFunctionType.Exp

    nc = bacc.Bacc("TRN2", target_bir_lowering=False, debug=False)
    # F32R-typed DRAM inputs: raw fp32 bits from the host; the PE rounds
    # f32r operands on the fly (verified on HW), and same-dtype DMA avoids
    # the SWDGE cast path (Q7 descriptor-gen was serializing the loads).
    HIDT = nc.dram_tensor("hidT", [E, S], F32R, kind="ExternalInput")
    WKV = nc.dram_tensor("wkv", [E, 2 * H], F32R, kind="ExternalInput")
    WQ = nc.dram_tensor("wq", [E, H], F32R, kind="ExternalInput")
    BKV = nc.dram_tensor("bkv", [2 * H, 1], F32, kind="ExternalInput")
    BQ = nc.dram_tensor("bq", [2 * H, 1], F32, kind="ExternalInput")
    OUT = nc.dram_tensor("out", [SQ, H], F32, kind="ExternalOutput")

    with tile.TileContext(nc) as tc, ExitStack() as ctx:
        consts = ctx.enter_context(tc.tile_pool(name="consts", bufs=1))
        hidp = ctx.enter_context(tc.tile_pool(name="hid", bufs=1))
        stage = ctx.enter_context(tc.tile_pool(name="stage", bufs=2))
        wtp = ctx.enter_context(tc.tile_pool(name="wt", bufs=4))
        pps = ctx.enter_context(tc.tile_pool(name="pps", bufs=2, space="PSUM"))
        sps = ctx.enter_context(tc.tile_pool(name="sps", bufs=2, space="PSUM"))
        avp = ctx.enter_context(tc.tile_pool(name="avp", bufs=2, space="PSUM"))

        # ---- constants ----
        wq_sb = consts.tile([128, NT, H], F32R)
        nc.gpsimd.dma_start(wq_sb[:], WQ[:].rearrange("(t p) c -> p t c", p=128))
        bq_sb = consts.tile([128, 1], F32)
        nc.gpsimd.dma_start(bq_sb[:], BQ[:])
        wkv_sb = consts.tile([128, NT, 2 * H], F32R)
        nc.gpsimd.dma_start(wkv_sb[:], WKV[:].rearrange("(t p) c -> p t c", p=128))
        bkv_sb = consts.tile([128, 1], F32)
        nc.gpsimd.dma_start(bkv_sb[:], BKV[:])
        identf = consts.tile([128, 128], F32)
        make_identity(nc, identf[:])
        identr = consts.tile([128, 128], F32R)
        nc.vector.tensor_copy(identr[:], identf[:])
        vones = consts.tile([128, NK, H + 1], F32R)
        ones32 = consts.tile([128, NK, 1], F32)
        nc.vector.memset(ones32[:], 1.0)
        nc.vector.tensor_copy(vones[:, :, 64:65], ones32[:])
        kT = consts.tile([128, S], F32R)
        qT = consts.tile([128, 2, 512], F32R)
        hidT_sb = hidp.tile([128, NT, S], F32R)

        # warm the Exp table early so the first real exp doesn't pay ~2.7us
        warm = consts.tile([1, 1], F32)
        nc.vector.memset(warm[:], 0.0)
        nc.scalar.activation(warm[:], warm[:], Exp)

        def dma_hid(c0, nch, t):
            nc.sync.dma_start(
                hidT_sb[:, t, 512 * c0 : 512 * (c0 + nch)],
                HIDT[128 * t : 128 * (t + 1), 512 * c0 : 512 * (c0 + nch)],
            )

        def q_group(rows, pcol, s0):
            pq = pps.tile([64, 512], F32, tag="pp")
            for t in range(NT):
                nc.tensor.matmul(
                    pq[:],
                    wq_sb[:, t, :],
                    hidT_sb[:, t, s0 : s0 + 512],
                    start=(t == 0),
                    stop=(t == NT - 1),
                )
            nc.vector.tensor_scalar_add(qT[rows, pcol, :], pq[:], bq_sb[0:64, :])

        def kv_chunk_pieces(c):
            box = {}

            def p_mm(t0, t1, start, stop):
                def fn():
                    if start:
                        box["pkv"] = pps.tile([128, 512], F32, tag="pp", name="pkv")
                    for t in range(t0, t1):
                        nc.tensor.matmul(
                            box["pkv"][:],
                            wkv_sb[:, t, :],
                            hidT_sb[:, t, 512 * c : 512 * (c + 1)],
                            start=(t == 0),
                            stop=(t == NT - 1),
                        )

                return fn

            def p_k():
                pkv = box["pkv"]
                cs = slice(512 * c, 512 * (c + 1))
                nc.vector.tensor_scalar_add(kT[0:64, cs], pkv[0:64, :], bkv_sb[0:64, :])
                nc.vector.tensor_scalar_add(
                    kT[64:128, cs], pkv[0:64, :], bkv_sb[0:64, :]
                )

            def p_v():
                pkv = box["pkv"]
                vstg = stage.tile([64, 512], F32R, tag="vstg")
                nc.vector.tensor_scalar_add(vstg[:], pkv[64:128, :], bkv_sb[64:128, :])
                box["vstg"] = vstg

            def p_tp():
                # 4 transposes into one psum tile, one batched copy to vones
                vstg = box["vstg"]
                pv = pps.tile([128, 4, 64], F32R, tag="pp")
                for j in range(4):
                    nc.tensor.transpose(
                        pv[:, j, :], vstg[:, 128 * j : 128 * (j + 1)], identr[0:64, 0:64]
                    )
                box["pv"] = pv

            def p_tpc():
                nc.vector.tensor_copy(vones[:, 4 * c : 4 * c + 4, 0:64], box["pv"][:])

            return [
                p_mm(0, 3, True, False),
                p_mm(3, 6, False, False),
                p_mm(6, 8, False, True),
                p_k,
                p_v,
                p_tp,
                p_tpc,
            ]

        pieces = []  # deque of deferred-work closures, popped per attn tile
        pend = []  # deferred AV matmuls: (wt, cols, av, t, t_first, t_last)

        def flush_pend():
            for wt, cols, av, t, t_first, t_last in pend:
                nc.tensor.matmul(
                    av[:],
                    vones[:, t, :],
                    wt[:, cols],
                    start=(t == t_first),
                    stop=(t == t_last),
                )
            pend.clear()

        def attn_tiles(P, avA, avB, ts, t_first, t_last, split=0, head=0, mid=None):
            ts = list(ts)
            scs = []
            for t in ts[:head]:
                # pipeline head: A-half score+exp only (chunk-0/q-group-1
                # deps); B halves follow once chunk 1 has landed, keeping
                # ACT fed while the B-side DMA completes
                sc = sps.tile([128, 1024], F32, tag="sc")
                nc.tensor.matmul(
                    sc[:, 0:512],
                    kT[0:64, 128 * t : 128 * (t + 1)],
                    qT[0:64, P, :],
                    start=True,
                    stop=True,
                )
                wt = wtp.tile([128, 1024], F32R, tag="wt")
                nc.scalar.activation(wt[:, 0:512], sc[:, 0:512], Exp, scale=0.125)
                pend.append((wt, slice(0, 512), avA, t, t_first, t_last))
                scs.append((t, sc, wt))
            if mid is not None:
                mid()
            for t, sc, wt in scs:
                nc.tensor.matmul(
                    sc[:, 512:1024],
                    kT[64:128, 128 * t : 128 * (t + 1)],
                    qT[64:128, P, :],
                    start=True,
                    stop=True,
                )
                nc.scalar.activation(wt[:, 512:1024], sc[:, 512:1024], Exp, scale=0.125)
                pend.append((wt, slice(512, 1024), avB, t, t_first, t_last))
            for t in ts[head:]:
                sc = sps.tile([128, 1024], F32, tag="sc")
                nc.tensor.matmul(
                    sc[:, 0:512],
                    kT[0:64, 128 * t : 128 * (t + 1)],
                    qT[0:64, P, :],
                    start=True,
                    stop=True,
                )
                wt = wtp.tile([128, 1024], F32R, tag="wt")
                if t - ts[0] < split + head:
                    # startup: exp the A half immediately (it only depends on
                    # kv chunk 0 / q group 1) so ACT starts ~5us earlier
                    nc.scalar.activation(
                        wt[:, 0:512], sc[:, 0:512], Exp, scale=0.125
                    )
                    pend.append((wt, slice(0, 512), avA, t, t_first, t_last))
                    nc.tensor.matmul(
                        sc[:, 512:1024],
                        kT[64:128, 128 * t : 128 * (t + 1)],
                        qT[64:128, P, :],
                        start=True,
                        stop=True,
                    )
                    nc.scalar.activation(
                        wt[:, 512:1024], sc[:, 512:1024], Exp, scale=0.125
                    )
                    pend.append((wt, slice(512, 1024), avB, t, t_first, t_last))
                else:
                    nc.tensor.matmul(
                        sc[:, 512:1024],
                        kT[64:128, 128 * t : 128 * (t + 1)],
                        qT[64:128, P, :],
                        start=True,
                        stop=True,
                    )
                    flush_pend()
                    nc.scalar.activation(wt[:], sc[:], Exp, scale=0.125)
                    pend.append((wt, slice(0, 512), avA, t, t_first, t_last))
                    pend.append((wt, slice(512, 1024), avB, t, t_first, t_last))
                i = t - ts[0]
                if head and i in (2, 3):
                    npop = 4
                else:
                    npop = 2 if len(pieces) > 27 else 1
                for _ in range(npop):
                    if pieces:
                        pieces.pop(0)()

        def out_block(av, acc, b):
            # combine the segment-spilled partial (SBUF) with the final psum
            ot = stage.tile([65, 512], F32, tag="ot")
            nc.vector.tensor_add(ot[:], av[:], acc[:])
            res = stage.tile([128, 4, H], F32, tag="res")
            for j in range(4):
                po = pps.tile([128, 65], F32, tag="pp")
                nc.tensor.transpose(
                    po[:], ot[:, 128 * j : 128 * (j + 1)], identf[0:65, 0:65]
                )
                rec = stage.tile([128, 1], F32, tag="rec")
                nc.vector.reciprocal(rec[:], po[:, 64:65])
                nc.vector.tensor_scalar_mul(res[:, j, :], po[:, 0:64], rec[:])
            nc.sync.dma_start(
                OUT[512 * b : 512 * (b + 1), :].rearrange("(j p) c -> p j c", p=128),
                res[:],
            )

        # ---- emission ----
        # Issue ALL hidT DMAs upfront: HWDGE streams them back-to-back, so
        # chunk c lands at ~bandwidth pace — ahead of the attention tile that
        # needs it.
        for c in range(8):
            for t in range(NT):
                dma_hid(c, 1, t)
        q_group(slice(0, 64), 0, 0)
        kv0 = kv_chunk_pieces(0)
        for fn in kv0[:4]:  # matmuls + kT copies; defer the v-transpose parts
            fn()
        q_group(slice(64, 128), 0, 512)

        # deferred compute, interleaved into the attention pipeline
        pieces += kv0[4:]
        pieces += kv_chunk_pieces(1)
        pieces += kv_chunk_pieces(2)
        pieces += kv_chunk_pieces(3)
        pieces += [
            lambda: q_group(slice(0, 64), 1, 1024),
            lambda: q_group(slice(64, 128), 1, 1536),
        ]
        for c in (4, 5, 6, 7):
            pieces += kv_chunk_pieces(c)

        # Attention runs as 4 interleaved 16-tile segments (P0a, P1a, P0b,
        # P1b) so projection pieces spread over the whole kernel instead of
        # front-loading pair 0. Each pair's first-half accumulator is spilled
        # to SBUF (only 2 PSUM banks available for AV accums).
        HK = NK // 2
        accs = {}

        def spill(av, key):
            acc = consts.tile([65, 512], F32, name=f"acc{key}")
            nc.vector.tensor_copy(acc[:], av[:])
            accs[key] = acc

        avA = avp.tile([65, 512], F32, tag="av", name="avA0a")
        avB = avp.tile([65, 512], F32, tag="av", name="avB0a")
        attn_tiles(0, avA, avB, range(0, HK), 0, HK - 1, split=16, head=2)
        flush_pend()
        spill(avA, "0A")
        spill(avB, "0B")

        avA = avp.tile([65, 512], F32, tag="av", name="avA1a")
        avB = avp.tile([65, 512], F32, tag="av", name="avB1a")
        attn_tiles(1, avA, avB, range(0, HK), 0, HK - 1, split=8)
        flush_pend()
        spill(avA, "1A")
        spill(avB, "1B")

        avA = avp.tile([65, 512], F32, tag="av", name="avA0b")
        avB = avp.tile([65, 512], F32, tag="av", name="avB0b")
        attn_tiles(0, avA, avB, range(HK, NK), HK, NK - 1)
        flush_pend()
        out_block(avA, accs["0A"], 0)
        out_block(avB, accs["0B"], 1)

        avA = avp.tile([65, 512], F32, tag="av", name="avA1b")
        avB = avp.tile([65, 512], F32, tag="av", name="avB1b")
        attn_tiles(1, avA, avB, range(HK, NK), HK, NK - 1)
        flush_pend()
        assert not pieces, f"{len(pieces)} deferred pieces never emitted"
        out_block(avA, accs["1A"], 2)
        out_block(avB, accs["1B"], 3)

    nc.compile()
    return nc


def kernel(hidden_states, Wq, bq, Wk, bk, Wv, bv):
    global _NC, LAST_RESULT
    from concourse.bass_utils import run_bass_kernel_spmd

    hidden_states = np.asarray(hidden_states, dtype=np.float32)
    Wq = np.asarray(Wq, dtype=np.float32)
    Wk = np.asarray(Wk, dtype=np.float32)
    Wv = np.asarray(Wv, dtype=np.float32)
    bq = np.asarray(bq, dtype=np.float32)
    bk = np.asarray(bk, dtype=np.float32)
    bv = np.asarray(bv, dtype=np.float32)
    B = hidden_states.shape[0]
    assert hidden_states.shape == (4, S, E), hidden_states.shape

    if _NC is None:
        _NC = _build()

    wkv = np.ascontiguousarray(np.concatenate([Wk, Wv], axis=1))
    bkv = np.concatenate([bk, bv]).reshape(2 * H, 1).copy()
    bq2 = np.concatenate([bq, bq]).reshape(2 * H, 1).copy()

    in_maps = []
    for core in range(N_CORES):
        b, half = divmod(core, 2)
        q0 = half * SQ
        hid_rot = np.roll(hidden_states[b], -q0, axis=0)
        in_maps.append(
            {
                "hidT": np.ascontiguousarray(hid_rot.T),
                "wkv": wkv,
                "wq": Wq,
                "bkv": bkv,
                "bq": bq2,
            }
        )

    LAST_RESULT = run_bass_kernel_spmd(_NC, in_maps, core_ids=list(range(N_CORES)))
    out = np.empty((B, S, H), np.float32)
    for core in range(N_CORES):
        b, half = divmod(core, 2)
        q0 = half * SQ
        out[b, q0 : q0 + SQ] = LAST_RESULT.results[core]["out"]
    return out

